# revision 8
# baseline (speedup 1.0000x reference)
"""DeepseekV3 decoder layer (MLA attention + MoE) on 8 Trainium2 NeuronCores.

Self-contained: kernel(**inputs) takes the full unsharded inputs and returns
the full [1, 1024, 2048] output.

Sharding:
  - token-parallel (128 tokens/core) for LN + latent projections, o-proj
    reduce-scatter, shared experts, final residual;
  - head-parallel (2 heads/core) for MLA attention over the full sequence
    (after an AllGather of the per-token latents);
  - expert-parallel (4 routed experts/core) for the MoE with on-device
    token gather/scatter (indirect DMA + DMA-transpose), ReduceScatter of
    the routed partial sums.
"""

import numpy as np
import ml_dtypes

import concourse.bass as bass
import concourse.tile as tile
import concourse.mybir as mybir
from concourse.bass_utils import run_bass_kernel_spmd

# ----------------------------------------------------------------------------
# Workaround for the pinned walrus build that rejects >1 sem wait per
# instruction.  Move extra waits onto single-wait InstNoOp carriers.
# ----------------------------------------------------------------------------
_SPLIT_COUNTER = [0]


def _split_waits_in_ordered(ordered):
    for _bb, insts in ordered.items():
        out = []
        changed = False
        for inst in insts:
            si = inst.sync_info
            ow = list(si.on_wait) if (si is not None and si.on_wait) else []
            if len(ow) > 1:
                changed = True
                for w in ow[:-1]:
                    _SPLIT_COUNTER[0] += 1
                    out.append(mybir.InstNoOp(
                        name=f"waitsplit-{_SPLIT_COUNTER[0]}",
                        engine=inst.engine, ins=[], outs=[],
                        sync_info=mybir.SyncInfo(on_wait=[w], on_update=[]),
                    ))
                inst.sync_info = mybir.SyncInfo(
                    on_wait=[ow[-1]],
                    on_update=list(si.on_update) if si.on_update else [],
                )
            out.append(inst)
        if changed:
            insts[:] = out
    return ordered


def _install_wait_split_patch():
    if getattr(tile.TileContext, "_waitsplit_installed", False):
        return
    _orig_lower = tile.TileContext._lower_ordered_insts

    def _patched_lower(self, ordered):
        _split_waits_in_ordered(ordered)
        return _orig_lower(self, ordered)

    def _patched_drain_and_barrier(self, tick_clock, wait_clock):
        nc = self.nc
        drain_inst = nc.sync.drain()
        wait_clock.add_sem_waits(
            drain_inst.ins, tile.ScopedClock({None: tick_clock.global_clock})
        )
        si = drain_inst.ins.sync_info
        ow = list(si.on_wait) if (si is not None and si.on_wait) else []
        if len(ow) > 1:
            drain_inst.ins.sync_info = mybir.SyncInfo(
                on_wait=[ow[0]],
                on_update=list(si.on_update) if si.on_update else [],
            )
            for w in ow[1:]:
                nop = nc.sync.nop(hint="waitsplit-drain", nofuse=True)
                nop.ins.sync_info = mybir.SyncInfo(on_wait=[w], on_update=[])
        nc.all_engine_barrier()
        assert self.sems is not None
        popped = nc._tile_sem_poison_stack.pop()
        assert popped is self._sem_poison
        nc.clear_and_free_semaphores(list(self.sems.allocated().values()))
        nc.all_engine_barrier()

    tile.TileContext._lower_ordered_insts = _patched_lower
    tile.TileContext._drain_and_barrier = _patched_drain_and_barrier
    tile.TileContext._waitsplit_installed = True


_install_wait_split_patch()

# ----------------------------------------------------------------------------
# Problem constants (hardcoded per the task contract)
# ----------------------------------------------------------------------------
EPS = 1e-5
H, S, B = 2048, 1024, 1
QR, KVR, RD, ND, VD, NH = 1536, 512, 64, 128, 128, 16
QKD = ND + RD                      # 192
E, I_FF, NSH = 32, 512, 2
NG, KG, TK = 8, 2, 8
GS = E // NG                       # 4

NC_N = 8                           # cores
TPC = S // NC_N                    # 128 tokens per core
HPC = NH // NC_N                   # 2 heads per core
EPC = E // NC_N                    # 4 experts per core
NMAX = 384                         # padded tokens per routed expert
NT = NMAX // 128                   # 3
NEG = -10000.0

F32 = mybir.dt.float32
BF16 = mybir.dt.bfloat16
I32 = mybir.dt.int32
AF = mybir.ActivationFunctionType
OP = mybir.AluOpType
AX = mybir.AxisListType
BF = ml_dtypes.bfloat16


# ----------------------------------------------------------------------------
# Device kernel builder
# ----------------------------------------------------------------------------
def _ln_raw(nc, pool, x_ap, width, tag):
    """Token-major raw layernorm: returns f32 tile (x-mu)/sqrt(var+eps).

    x_ap: [128, width] f32 (SBUF or PSUM slice)."""
    mu = pool.tile([128, 1], F32, tag=f"{tag}_mu")
    nc.vector.reduce_sum(out=mu[:], in_=x_ap, axis=AX.X)
    nc.vector.tensor_scalar(out=mu[:], in0=mu[:], scalar1=1.0 / width, scalar2=None, op0=OP.mult)
    xm = pool.tile([128, width], F32, tag=f"{tag}_xm")
    nc.vector.tensor_scalar(out=xm[:], in0=x_ap, scalar1=mu[:, :1], scalar2=None, op0=OP.subtract)
    var = pool.tile([128, 1], F32, tag=f"{tag}_var")
    sq = pool.tile([128, width], F32, tag=f"{tag}_sq")
    nc.scalar.activation(sq[:], xm[:], AF.Square, accum_out=var[:])
    nc.vector.tensor_scalar(out=var[:], in0=var[:], scalar1=1.0 / width, scalar2=EPS, op0=OP.mult, op1=OP.add)
    sd = pool.tile([128, 1], F32, tag=f"{tag}_sd")
    nc.scalar.activation(sd[:], var[:], AF.Sqrt)
    rstd = pool.tile([128, 1], F32, tag=f"{tag}_rstd")
    nc.vector.reciprocal(rstd[:], sd[:])
    # one Newton refinement: r' = r*(1.5 - 0.5*var*r^2)
    t = pool.tile([128, 1], F32, tag=f"{tag}_nt")
    nc.vector.tensor_tensor(out=t[:], in0=rstd[:], in1=rstd[:], op=OP.mult)
    nc.vector.tensor_tensor(out=t[:], in0=t[:], in1=var[:], op=OP.mult)
    nc.vector.tensor_scalar(out=t[:], in0=t[:], scalar1=-0.5, scalar2=1.5, op0=OP.mult, op1=OP.add)
    nc.vector.tensor_tensor(out=rstd[:], in0=rstd[:], in1=t[:], op=OP.mult)
    out = pool.tile([128, width], F32, tag=f"{tag}_out")
    nc.vector.tensor_scalar(out=out[:], in0=xm[:], scalar1=rstd[:, :1], scalar2=None, op0=OP.mult)
    return out


def build_nc():
    import contextlib

    nc = bass.Bass()
    dp = nc.declare_dram_parameter

    # per-core inputs (content differs per core; shapes identical)
    x_in = dp("x", [TPC, H], F32, isOutput=False)
    cs_loc = dp("cs_loc", [TPC, 2 * RD], F32, isOutput=False)       # [cos|sin] local tokens
    wqb_nT = dp("wqb_nT", [QR, HPC, ND], BF16, isOutput=False)
    wqb_rT = dp("wqb_rT", [QR, HPC, RD], BF16, isOutput=False)
    wkv_kT = dp("wkv_kT", [KVR, HPC, ND], BF16, isOutput=False)
    wkv_v = dp("wkv_v", [KVR, HPC, VD], BF16, isOutput=False)
    woT = dp("woT", [HPC * VD, H], BF16, isOutput=False)
    egu = dp("egu", [EPC, H, 2 * I_FF], BF16, isOutput=False)
    edown = dp("edown", [EPC, I_FF, H], BF16, isOutput=False)
    emask4 = dp("emask4", [EPC, 128, E], F32, isOutput=False)
    # replicated inputs
    wqkvT = dp("wqkvT", [H, QR + KVR + RD], BF16, isOutput=False)
    cosT_in = dp("cosT", [RD, S], F32, isOutput=False)              # feature-major rope tables
    sinT_in = dp("sinT", [RD, S], F32, isOutput=False)
    gate_wT = dp("gate_wT", [H, E], F32, isOutput=False)
    sgu = dp("sgu", [NSH, H, 2 * I_FF], BF16, isOutput=False)
    sdown = dp("sdown", [NSH, I_FF, H], BF16, isOutput=False)
    tri_in = dp("tri", [128, 128], F32, isOutput=False)
    ones_in = dp("ones", [128, 128], F32, isOutput=False)
    identb_in = dp("identb", [128, 128], BF16, isOutput=False)
    identf_in = dp("identf", [128, 128], F32, isOutput=False)
    cmask_in = dp("cmask", [128, 128], F32, isOutput=False)
    ids_in = dp("ids", [128, S // 128], F32, isOutput=False)

    out_t = dp("out", [TPC, H], F32, isOutput=True)

    KC = H // 128          # 16
    QKC = QR // 128        # 12
    KVC = KVR // 128       # 4
    STL = S // 128         # 8 sequence tiles

    rg = [list(range(NC_N))]

    with tile.TileContext(nc) as tc, contextlib.ExitStack() as top:
        const = top.enter_context(tc.tile_pool(name="const", bufs=1))
        dram = top.enter_context(tc.tile_pool(name="dram", bufs=1, space="DRAM"))

        bc_nmax = nc.gpsimd.to_reg(NMAX - 1)
        bc_s = nc.gpsimd.to_reg(S - 1)
        tri = const.tile([128, 128], F32); nc.sync.dma_start(tri[:], tri_in[:])
        ones = const.tile([128, 128], F32); nc.sync.dma_start(ones[:], ones_in[:])
        identb = const.tile([128, 128], BF16); nc.sync.dma_start(identb[:], identb_in[:])
        identf = const.tile([128, 128], F32); nc.sync.dma_start(identf[:], identf_in[:])
        cmask = const.tile([128, 128], F32); nc.sync.dma_start(cmask[:], cmask_in[:])
        ids8 = const.tile([128, S // 128], F32); nc.sync.dma_start(ids8[:], ids_in[:])

        # DRAM buffers
        ag_lat_in = dram.tile([TPC, QR + KVR + RD], BF16)
        lat_full = dram.tile([S, QR + KVR + RD], BF16)
        ypart = dram.tile([S, H], BF16)
        y_rs = dram.tile([TPC, H], BF16)
        h2_bf_dram = dram.tile([TPC, H], BF16)
        h2_full = dram.tile([S, H], BF16)
        comb_loc = dram.tile([TPC, E], F32)
        comb_full = dram.tile([S, E], F32)
        routed_dram = dram.tile([S, H], BF16)
        routed_rs = dram.tile([TPC, H], BF16)
        g_drams = [dram.tile([NMAX, H], BF16, name=f"g_dram{_e}") for _e in range(EPC)]
        ew_drams = [dram.tile([NMAX, 2], F32, name=f"ew_dram{_e}") for _e in range(EPC)]

        # persistent SBUF
        x_sb = const.tile([128, H], F32)
        nc.sync.dma_start(x_sb[:], x_in[:])
        x2_sb = const.tile([128, H], F32)       # x + attn
        out_acc = const.tile([128, H], F32)     # x2 + shared (+ routed at end)
        h2_sb = const.tile([128, H], F32)
        h2T_f = const.tile([128, KC, 128], F32)   # feature-major local h2 (f32)
        h2T_b = const.tile([128, KC, 128], BF16)  # same, bf16

        # zero the big accumulators early (overlaps with compute)
        zero_bf = const.tile([128, H], BF16)
        nc.vector.memset(zero_bf[:], 0.0)
        for tt in range(STL):
            nc.gpsimd.dma_start(routed_dram[tt * 128:(tt + 1) * 128, :], zero_bf[:])
        for e in range(EPC):
            for t3 in range(NT):
                nc.gpsimd.dma_start(g_drams[e][t3 * 128:(t3 + 1) * 128, :], zero_bf[:])
        zero_sm = const.tile([128, NT * 2], F32)
        nc.vector.memset(zero_sm[:], 0.0)
        for e in range(EPC):
            nc.gpsimd.dma_start(ew_drams[e][:].rearrange("(a p) b -> p a b", p=128),
                                zero_sm[:].rearrange("p (a b) -> p a b", b=2))

        # ------------------------------------------------------------------
        # Phase A: LN1, latent projections, rope(k), AllGather latents
        # ------------------------------------------------------------------
        with contextlib.ExitStack() as ctx:
            pa = ctx.enter_context(tc.tile_pool(name="pa", bufs=2))
            paw = ctx.enter_context(tc.tile_pool(name="paw", bufs=3))
            pap = ctx.enter_context(tc.tile_pool(name="pap", bufs=1, space="PSUM"))
            papT = ctx.enter_context(tc.tile_pool(name="papT", bufs=2, space="PSUM"))

            h_f = _ln_raw(nc, pa, x_sb[:], H, "ln1")
            h_b = pa.tile([128, H], BF16, tag="h_b")
            nc.vector.tensor_copy(out=h_b[:], in_=h_f[:])
            hT = pa.tile([128, KC, 128], BF16, tag="hT")
            for kc in range(KC):
                tp = papT.tile([128, 128], BF16, tag="hT_tp")
                nc.tensor.transpose(tp[:], h_b[:, kc * 128:(kc + 1) * 128], identb[:])
                nc.vector.tensor_copy(out=hT[:, kc, :], in_=tp[:])

            # qkv = h @ wqkvT : out [128 tok, 2112] in 5 psum groups
            NW = QR + KVR + RD   # 2112
            qkv_ps = [pap.tile([128, min(512, NW - n * 512)], F32, name=f"qkv{n}", tag=f"qkv{n}")
                      for n in range((NW + 511) // 512)]
            for kc in range(KC):
                wt = paw.tile([128, NW], BF16, tag="wqkv_t")
                nc.sync.dma_start(wt[:], wqkvT[kc * 128:(kc + 1) * 128, :])
                for n, ps in enumerate(qkv_ps):
                    w = ps.shape[1]
                    nc.tensor.matmul(ps[:], hT[:, kc, :], wt[:, n * 512:n * 512 + w],
                                     start=(kc == 0), stop=(kc == KC - 1))
            qkv = pa.tile([128, NW], F32, tag="qkv")
            for n, ps in enumerate(qkv_ps):
                w = ps.shape[1]
                nc.scalar.activation(qkv[:, n * 512:n * 512 + w], ps[:], AF.Copy)

            qlat = _ln_raw(nc, pa, qkv[:, 0:QR], QR, "lnqa")
            kvlat = _ln_raw(nc, pa, qkv[:, QR:QR + KVR], KVR, "lnkv")

            # rope k (token-major, local positions)
            cs = pa.tile([128, 2 * RD], F32, tag="cs")
            nc.sync.dma_start(cs[:], cs_loc[:])
            kr = qkv[:, QR + KVR:QR + KVR + RD]
            h_half = RD // 2
            kro = pa.tile([128, RD], F32, tag="kro")
            t_a = pa.tile([128, h_half], F32, tag="kr_a")
            # out[0:32] = kr[0:32]*cos[0:32] - kr[32:64]*sin[0:32]
            nc.vector.tensor_tensor(out=kro[:, 0:h_half], in0=kr[:, 0:h_half], in1=cs[:, 0:h_half], op=OP.mult)
            nc.vector.tensor_tensor(out=t_a[:], in0=kr[:, h_half:RD], in1=cs[:, RD:RD + h_half], op=OP.mult)
            nc.vector.tensor_tensor(out=kro[:, 0:h_half], in0=kro[:, 0:h_half], in1=t_a[:], op=OP.subtract)
            # out[32:64] = kr[32:64]*cos[32:64] + kr[0:32]*sin[32:64]
            nc.vector.tensor_tensor(out=kro[:, h_half:RD], in0=kr[:, h_half:RD], in1=cs[:, h_half:RD], op=OP.mult)
            nc.vector.tensor_tensor(out=t_a[:], in0=kr[:, 0:h_half], in1=cs[:, RD + h_half:2 * RD], op=OP.mult)
            nc.vector.tensor_tensor(out=kro[:, h_half:RD], in0=kro[:, h_half:RD], in1=t_a[:], op=OP.add)

            agb = pa.tile([128, NW], BF16, tag="agb")
            nc.vector.tensor_copy(out=agb[:, 0:QR], in_=qlat[:])
            nc.vector.tensor_copy(out=agb[:, QR:QR + KVR], in_=kvlat[:])
            nc.vector.tensor_copy(out=agb[:, QR + KVR:NW], in_=kro[:])
            nc.sync.dma_start(ag_lat_in[:], agb[:])
            nc.gpsimd.collective_compute(
                "AllGather", OP.bypass, replica_groups=rg,
                ins=[ag_lat_in[:].opt()], outs=[lat_full[:].opt()])

        # ------------------------------------------------------------------
        # Phase B+C+D: attention for this core's 2 heads + o-proj partial + RS
        # ------------------------------------------------------------------
        with contextlib.ExitStack() as ctx:
            pb = ctx.enter_context(tc.tile_pool(name="pb", bufs=2))
            pbl = ctx.enter_context(tc.tile_pool(name="pbl", bufs=1))
            pbp = ctx.enter_context(tc.tile_pool(name="pbp", bufs=2, space="PSUM"))
            pbs = ctx.enter_context(tc.tile_pool(name="pbs", bufs=1, space="PSUM"))

            # feature-major full-sequence latents
            qlatT = pbl.tile([128, QKC, S], BF16, tag="qlatT")
            for s_ in range(QKC):
                nc.sync.dma_start_transpose(qlatT[:, s_, :], lat_full[:, s_ * 128:(s_ + 1) * 128])
            kvlatT = pbl.tile([128, KVC, S], BF16, tag="kvlatT")
            for s_ in range(KVC):
                nc.sync.dma_start_transpose(kvlatT[:, s_, :], lat_full[:, QR + s_ * 128:QR + (s_ + 1) * 128])
            # k-rope feature-major via PE transposes
            kropeT = pbl.tile([RD, S], BF16, tag="kropeT")
            for tt in range(STL):
                krt = pb.tile([128, RD], BF16, tag="kr_tm")
                nc.sync.dma_start(krt[:], lat_full[tt * 128:(tt + 1) * 128, QR + KVR:NW])
                tp = pbp.tile([RD, 128], BF16, tag="ps512")
                nc.tensor.transpose(tp[:], krt[:], identb[:])
                nc.vector.tensor_copy(out=kropeT[:, tt * 128:(tt + 1) * 128], in_=tp[:])
            cosT_sb = pbl.tile([RD, S], F32, tag="cosT")
            nc.sync.dma_start(cosT_sb[:], cosT_in[:])
            sinT_sb = pbl.tile([RD, S], F32, tag="sinT")
            nc.sync.dma_start(sinT_sb[:], sinT_in[:])

            oT = [pbl.tile([128, S], BF16, name=f"oT{_h}") for _h in range(HPC)]

            for h in range(HPC):
                # q nope/rope (feature-major over all tokens)
                qnT = pbl.tile([ND, S], BF16, tag="qnT")
                qrT = pbl.tile([RD, S], BF16, tag="qrT")
                wqn = pb.tile([128, QKC, ND], BF16, tag="wqn")
                nc.sync.dma_start(wqn[:], wqb_nT[:, h, :].rearrange("(c p) d -> p c d", p=128))
                wqr = pb.tile([128, QKC, RD], BF16, tag="wqr")
                nc.sync.dma_start(wqr[:], wqb_rT[:, h, :].rearrange("(c p) d -> p c d", p=128))
                for n in range(S // 512):
                    psn = pbp.tile([ND, 512], F32, tag="ps512")
                    psr = pbp.tile([RD, 512], F32, tag="ps_r")
                    for kc in range(QKC):
                        rhs = qlatT[:, kc, n * 512:(n + 1) * 512]
                        nc.tensor.matmul(psn[:], wqn[:, kc, :], rhs, start=(kc == 0), stop=(kc == QKC - 1))
                        nc.tensor.matmul(psr[:], wqr[:, kc, :], rhs, start=(kc == 0), stop=(kc == QKC - 1))
                    nc.vector.tensor_copy(out=qnT[:, n * 512:(n + 1) * 512], in_=psn[:])
                    # rope q: rot(x)[d] = -x[d+32] (d<32) / x[d-32] (d>=32),
                    # built with partition-shift DMAs so all DVE ops are
                    # partition-aligned (walrus verifier requirement).
                    hh = RD // 2
                    qr_raw = pb.tile([RD, 512], F32, tag="qr_raw")
                    nc.vector.tensor_copy(out=qr_raw[:], in_=psr[:])
                    rot = pb.tile([RD, 512], F32, tag="rot")
                    nc.sync.dma_start(rot[0:hh, :], qr_raw[hh:RD, :])
                    nc.sync.dma_start(rot[hh:RD, :], qr_raw[0:hh, :])
                    nc.vector.tensor_scalar(out=rot[0:hh, :], in0=rot[0:hh, :], scalar1=-1.0,
                                            scalar2=None, op0=OP.mult)
                    qr_f = pb.tile([RD, 512], F32, tag="qr_f")
                    nc.vector.tensor_tensor(out=qr_f[:], in0=qr_raw[:],
                                            in1=cosT_sb[:, n * 512:(n + 1) * 512], op=OP.mult)
                    nc.vector.tensor_tensor(out=rot[:], in0=rot[:],
                                            in1=sinT_sb[:, n * 512:(n + 1) * 512], op=OP.mult)
                    nc.vector.tensor_tensor(out=qr_f[:], in0=qr_f[:], in1=rot[:], op=OP.add)
                    nc.vector.tensor_copy(out=qrT[:, n * 512:(n + 1) * 512], in_=qr_f[:])

                # k nope feature-major
                knT = pbl.tile([ND, S], BF16, tag="knT")
                wkk = pb.tile([128, KVC, ND], BF16, tag="wkk")
                nc.sync.dma_start(wkk[:], wkv_kT[:, h, :].rearrange("(c p) d -> p c d", p=128))
                for n in range(S // 512):
                    psk = pbp.tile([ND, 512], F32, tag="ps512")
                    for kc in range(KVC):
                        nc.tensor.matmul(psk[:], wkk[:, kc, :], kvlatT[:, kc, n * 512:(n + 1) * 512],
                                         start=(kc == 0), stop=(kc == KVC - 1))
                    nc.vector.tensor_copy(out=knT[:, n * 512:(n + 1) * 512], in_=psk[:])

                # v token-major
                v_tm = pbl.tile([128, STL, VD], BF16, tag="v_tm")
                wvv = pb.tile([128, KVC, VD], BF16, tag="wvv")
                nc.sync.dma_start(wvv[:], wkv_v[:, h, :].rearrange("(c p) d -> p c d", p=128))
                for tt in range(STL):
                    psv = pbp.tile([128, VD], F32, tag="ps512")
                    for kc in range(KVC):
                        nc.tensor.matmul(psv[:], kvlatT[:, kc, tt * 128:(tt + 1) * 128], wvv[:, kc, :],
                                         start=(kc == 0), stop=(kc == KVC - 1))
                    nc.vector.tensor_copy(out=v_tm[:, tt, :], in_=psv[:])

                # attention per query tile
                for tq in range(STL):
                    W = (tq + 1) * 128
                    s_ps = pbs.tile([128, 1024], F32, tag="s_ps")
                    for nch in range((W + 511) // 512):
                        wdt = min(512, W - nch * 512)
                        sl = slice(nch * 512, nch * 512 + wdt)
                        nc.tensor.matmul(s_ps[:, sl], qnT[:, tq * 128:(tq + 1) * 128], knT[:, sl],
                                         start=True, stop=False)
                        nc.tensor.matmul(s_ps[:, sl], qrT[:, tq * 128:(tq + 1) * 128], kropeT[:, sl],
                                         start=False, stop=True)
                    # scale by 1/sqrt(QKD) applied via exp scale arg below
                    nc.vector.tensor_tensor(out=s_ps[:, tq * 128:W], in0=s_ps[:, tq * 128:W],
                                            in1=cmask[:], op=OP.add)
                    p_f = pb.tile([128, 1024], F32, tag="p_f")
                    den = pb.tile([128, 1], F32, tag="den")
                    nc.scalar.activation(p_f[:, 0:W], s_ps[:, 0:W], AF.Exp,
                                         scale=float(1.0 / np.sqrt(QKD)), accum_out=den[:])
                    rden = pb.tile([128, 1], F32, tag="rden")
                    nc.vector.reciprocal(rden[:], den[:])
                    p_b = pb.tile([128, 1024], BF16, tag="p_b")
                    nc.vector.tensor_scalar(out=p_b[:, 0:W], in0=p_f[:, 0:W],
                                            scalar1=rden[:, :1], scalar2=None, op0=OP.mult)
                    o_ps = pbp.tile([128, 128], F32, tag="ps_o")
                    for kb in range(tq + 1):
                        ptp = pbp.tile([128, 128], BF16, tag="ps512")
                        nc.tensor.transpose(ptp[:], p_b[:, kb * 128:(kb + 1) * 128], identb[:])
                        pT = pb.tile([128, 128], BF16, tag="pT")
                        nc.vector.tensor_copy(out=pT[:], in_=ptp[:])
                        nc.tensor.matmul(o_ps[:], v_tm[:, kb, :], pT[:],
                                         start=(kb == 0), stop=(kb == tq))
                    nc.vector.tensor_copy(out=oT[h][:, tq * 128:(tq + 1) * 128], in_=o_ps[:])

            # o-proj partial: ypart[t, :] = sum_h oT_h.T @ woT[h*128:(h+1)*128+...]
            wo_sb = pbl.tile([128, HPC, H], BF16, tag="wo_sb")
            nc.sync.dma_start(wo_sb[:], woT[:].rearrange("(c p) d -> p c d", p=128))
            for tt in range(STL):
                yrow = pb.tile([128, H], BF16, tag="yrow")
                for n in range(H // 512):
                    yps = pbp.tile([128, 512], F32, tag="ps512")
                    for h in range(HPC):
                        nc.tensor.matmul(yps[:], oT[h][:, tt * 128:(tt + 1) * 128],
                                         wo_sb[:, h, n * 512:(n + 1) * 512],
                                         start=(h == 0), stop=(h == HPC - 1))
                    nc.scalar.activation(yrow[:, n * 512:(n + 1) * 512], yps[:], AF.Copy)
                nc.sync.dma_start(ypart[tt * 128:(tt + 1) * 128, :], yrow[:])
            nc.gpsimd.collective_compute(
                "ReduceScatter", OP.add, replica_groups=rg,
                ins=[ypart[:].opt()], outs=[y_rs[:].opt()])

        # ------------------------------------------------------------------
        # Phase E: residual, LN2, gate logits, grouped top-k routing
        # ------------------------------------------------------------------
        with contextlib.ExitStack() as ctx:
            pe = ctx.enter_context(tc.tile_pool(name="pe", bufs=2))
            pep = ctx.enter_context(tc.tile_pool(name="pep", bufs=2, space="PSUM"))

            yb = pe.tile([128, H], BF16, tag="yb")
            nc.sync.dma_start(yb[:], y_rs[:])
            nc.vector.tensor_tensor(out=x2_sb[:], in0=x_sb[:], in1=yb[:], op=OP.add)
            h2v = _ln_raw(nc, pe, x2_sb[:], H, "ln2")
            nc.vector.tensor_copy(out=h2_sb[:], in_=h2v[:])
            h2b = pe.tile([128, H], BF16, tag="h2b")
            nc.vector.tensor_copy(out=h2b[:], in_=h2_sb[:])
            nc.sync.dma_start(h2_bf_dram[:], h2b[:])
            nc.gpsimd.collective_compute(
                "AllGather", OP.bypass, replica_groups=rg,
                ins=[h2_bf_dram[:].opt()], outs=[h2_full[:].opt()])

            for kc in range(KC):
                tpf = pep.tile([128, 128], F32, tag="h2T_tp")
                nc.tensor.transpose(tpf[:], h2_sb[:, kc * 128:(kc + 1) * 128], identf[:])
                nc.vector.tensor_copy(out=h2T_f[:, kc, :], in_=tpf[:])
                nc.vector.tensor_copy(out=h2T_b[:, kc, :], in_=tpf[:])

            gw = pe.tile([128, KC, E], F32, tag="gw")
            nc.sync.dma_start(gw[:], gate_wT[:].rearrange("(c p) e -> p c e", p=128))
            lg_ps = pep.tile([128, E], F32, tag="lg_ps")
            for kc in range(KC):
                nc.tensor.matmul(lg_ps[:], h2T_f[:, kc, :], gw[:, kc, :],
                                 start=(kc == 0), stop=(kc == KC - 1))
            lg = pe.tile([128, E], F32, tag="lg")
            nc.vector.tensor_copy(out=lg[:], in_=lg_ps[:])

            # grouped top-2 then top-8 threshold selection
            lg3 = lg[:].rearrange("p (g s) -> p g s", s=GS)
            m12 = pe.tile([128, 2 * NG], F32, tag="m12")
            nc.vector.reduce_max(out=m12[:, 0:NG], in_=lg3, axis=AX.X)
            m1b = m12[:, 0:NG].rearrange("p (g o) -> p g o", o=1).to_broadcast([128, NG, GS])
            eq = pe.tile([128, E], F32, tag="eq")
            nc.vector.tensor_tensor(out=eq[:].rearrange("p (g s) -> p g s", s=GS), in0=lg3, in1=m1b, op=OP.is_equal)
            masked = pe.tile([128, E], F32, tag="masked")
            nc.vector.tensor_scalar(out=masked[:], in0=eq[:], scalar1=NEG, scalar2=None, op0=OP.mult)
            nc.vector.tensor_tensor(out=masked[:], in0=lg[:], in1=masked[:], op=OP.add)
            nc.vector.reduce_max(out=m12[:, NG:2 * NG],
                                 in_=masked[:].rearrange("p (g s) -> p g s", s=GS), axis=AX.X)
            top8 = pe.tile([128, 8], F32, tag="top8")
            nc.vector.max(out=top8[:], in_=m12[:])
            thr = pe.tile([128, 1], F32, tag="thr")
            nc.vector.tensor_reduce(out=thr[:], in_=top8[:], op=OP.min, axis=AX.X)
            ge_thr = pe.tile([128, E], F32, tag="ge_thr")
            nc.vector.tensor_scalar(out=ge_thr[:], in0=lg[:], scalar1=thr[:, :1], scalar2=None, op0=OP.is_ge)
            m2b = m12[:, NG:2 * NG].rearrange("p (g o) -> p g o", o=1).to_broadcast([128, NG, GS])
            ge_m2 = pe.tile([128, E], F32, tag="ge_m2")
            nc.vector.tensor_tensor(out=ge_m2[:].rearrange("p (g s) -> p g s", s=GS), in0=lg3, in1=m2b, op=OP.is_ge)
            sel = pe.tile([128, E], F32, tag="sel")
            nc.vector.tensor_tensor(out=sel[:], in0=ge_thr[:], in1=ge_m2[:], op=OP.mult)
            wraw = pe.tile([128, E], F32, tag="wraw")
            nc.vector.tensor_tensor(out=wraw[:], in0=lg[:], in1=sel[:], op=OP.mult)
            rsum = pe.tile([128, 1], F32, tag="rsum")
            nc.vector.reduce_sum(out=rsum[:], in_=wraw[:], axis=AX.X)
            nc.vector.tensor_scalar(out=rsum[:], in0=rsum[:], scalar1=1e-20, scalar2=None, op0=OP.add)
            rden2 = pe.tile([128, 1], F32, tag="rden2")
            nc.vector.reciprocal(rden2[:], rsum[:])
            combt = pe.tile([128, E], F32, tag="combt")
            nc.vector.tensor_scalar(out=combt[:], in0=wraw[:], scalar1=rden2[:, :1], scalar2=None, op0=OP.mult)
            nc.sync.dma_start(comb_loc[:], combt[:])
            nc.gpsimd.collective_compute(
                "AllGather", OP.bypass, replica_groups=rg,
                ins=[comb_loc[:].opt()], outs=[comb_full[:].opt()])

        # ------------------------------------------------------------------
        # Phase F: MoE expert-parallel: list build + expert MLPs + scatter-add
        # ------------------------------------------------------------------
        with contextlib.ExitStack() as ctx:
            pf = ctx.enter_context(tc.tile_pool(name="pf", bufs=2))
            pfe = ctx.enter_context(tc.tile_pool(name="pfe", bufs=2))
            pfw = ctx.enter_context(tc.tile_pool(name="pfw", bufs=2))
            pfp = ctx.enter_context(tc.tile_pool(name="pfp", bufs=1, space="PSUM"))
            pfpT = ctx.enter_context(tc.tile_pool(name="pfpT", bufs=2, space="PSUM"))
            pfpY = ctx.enter_context(tc.tile_pool(name="pfpY", bufs=2, space="PSUM"))

            emk = pf.tile([128, EPC, E], F32, tag="emk")
            nc.sync.dma_start(emk[:], emask4[:].rearrange("e p c -> p e c"))

            sel_run = pf.tile([128, EPC], F32, tag="sel_run")
            nc.vector.memset(sel_run[:], 0.0)
            for tt in range(STL):
                cbt = pf.tile([128, E], F32, tag="cbt")
                nc.sync.dma_start(cbt[:], comb_full[tt * 128:(tt + 1) * 128, :])
                h2t = pf.tile([128, H], BF16, tag="h2t")
                nc.sync.dma_start(h2t[:], h2_full[tt * 128:(tt + 1) * 128, :])
                wcol = pf.tile([128, EPC], F32, tag="wcol")
                selc = pf.tile([128, EPC], F32, tag="selc")
                for e in range(EPC):
                    wm = pf.tile([128, E], F32, tag="wm")
                    nc.vector.tensor_tensor(out=wm[:], in0=cbt[:], in1=emk[:, e, :], op=OP.mult)
                    nc.vector.reduce_sum(out=wcol[:, e:e + 1], in_=wm[:], axis=AX.X)
                nc.vector.tensor_scalar(out=selc[:], in0=wcol[:], scalar1=0.0, scalar2=None, op0=OP.not_equal)
                cum_ps = pfp.tile([128, EPC], F32, tag="cum")
                nc.tensor.matmul(cum_ps[:], tri[:], selc[:], start=True, stop=(tt == 0))
                if tt > 0:
                    nc.tensor.matmul(cum_ps[:], ones[:], sel_run[:], start=False, stop=True)
                nc.vector.tensor_tensor(out=sel_run[:], in0=sel_run[:], in1=selc[:], op=OP.add)
                dest = pf.tile([128, EPC], F32, tag="dest")
                nc.vector.tensor_scalar(out=dest[:], in0=cum_ps[:], scalar1=-1.0, scalar2=None, op0=OP.add)
                nc.vector.tensor_tensor(out=dest[:], in0=dest[:], in1=selc[:], op=OP.mult)
                t2 = pf.tile([128, EPC], F32, tag="t2")
                nc.vector.tensor_scalar(out=t2[:], in0=selc[:], scalar1=-1e6, scalar2=1e6, op0=OP.mult, op1=OP.add)
                nc.vector.tensor_tensor(out=dest[:], in0=dest[:], in1=t2[:], op=OP.add)
                dest_i = pf.tile([128, EPC], I32, tag="dest_i")
                nc.vector.tensor_copy(out=dest_i[:], in_=dest[:])
                for e in range(EPC):
                    pack = pf.tile([128, 2], F32, tag="pack")
                    nc.vector.tensor_copy(out=pack[:, 0:1], in_=ids8[:, tt:tt + 1])
                    nc.vector.tensor_copy(out=pack[:, 1:2], in_=wcol[:, e:e + 1])
                    nc.gpsimd.indirect_dma_start(
                        out=ew_drams[e][:],
                        out_offset=bass.IndirectOffsetOnAxis(ap=dest_i[:, e:e + 1], axis=0),
                        in_=pack[:], in_offset=None,
                        bounds_check=bc_nmax, oob_is_err=False)
                    nc.gpsimd.indirect_dma_start(
                        out=g_drams[e][:],
                        out_offset=bass.IndirectOffsetOnAxis(ap=dest_i[:, e:e + 1], axis=0),
                        in_=h2t[:], in_offset=None,
                        bounds_check=bc_nmax, oob_is_err=False)

            for e in range(EPC):
                xt = pfe.tile([128, KC, NMAX], BF16, tag="xt")
                for s_ in range(KC):
                    nc.sync.dma_start_transpose(xt[:, s_, :], g_drams[e][:, s_ * 128:(s_ + 1) * 128])
                idlf = pf.tile([128, NT], F32, tag="idlf")
                nc.sync.dma_start(idlf[:], ew_drams[e][:, 0:1].rearrange("(c p) one -> p (c one)", p=128))
                idl = pf.tile([128, NT], I32, tag="idl")
                nc.vector.tensor_copy(out=idl[:], in_=idlf[:])
                wl = pf.tile([128, NT], F32, tag="wl")
                nc.sync.dma_start(wl[:], ew_drams[e][:, 1:2].rearrange("(c p) one -> p (c one)", p=128))

                guT_sb = pfw.tile([128, KC, 2 * I_FF], BF16, tag="guT")
                nc.sync.dma_start(guT_sb[:], egu[e].rearrange("(c p) f -> p c f", p=128))
                dT_sb = pfw.tile([128, I_FF // 128, H], BF16, tag="dT")
                nc.sync.dma_start(dT_sb[:], edown[e].rearrange("(c p) f -> p c f", p=128))

                for t3 in range(NT):
                    g1 = pfp.tile([128, I_FF], F32, tag="g1")
                    g2 = pfp.tile([128, I_FF], F32, tag="g2")
                    for kc in range(KC):
                        lhsT = xt[:, kc, t3 * 128:(t3 + 1) * 128]
                        nc.tensor.matmul(g1[:], lhsT, guT_sb[:, kc, 0:I_FF],
                                         start=(kc == 0), stop=(kc == KC - 1))
                        nc.tensor.matmul(g2[:], lhsT, guT_sb[:, kc, I_FF:2 * I_FF],
                                         start=(kc == 0), stop=(kc == KC - 1))
                    sig = pf.tile([128, I_FF], F32, tag="sig")
                    nc.scalar.activation(sig[:], g1[:], AF.Sigmoid)
                    sil = pf.tile([128, I_FF], F32, tag="sil")
                    nc.vector.tensor_tensor(out=sil[:], in0=sig[:], in1=g1[:], op=OP.mult)
                    a_bf = pf.tile([128, I_FF], BF16, tag="a_bf")
                    nc.vector.tensor_tensor(out=a_bf[:], in0=sil[:], in1=g2[:], op=OP.mult)
                    aT = pf.tile([128, I_FF // 128, 128], BF16, tag="aT")
                    for j in range(I_FF // 128):
                        tp = pfpT.tile([128, 128], BF16, tag="a_tp")
                        nc.tensor.transpose(tp[:], a_bf[:, j * 128:(j + 1) * 128], identb[:])
                        nc.vector.tensor_copy(out=aT[:, j, :], in_=tp[:])
                    y_sc = pf.tile([128, H], BF16, tag="y_sc")
                    for n in range(H // 512):
                        yp = pfpY.tile([128, 512], F32, tag="yp")
                        for kc in range(I_FF // 128):
                            nc.tensor.matmul(yp[:], aT[:, kc, :], dT_sb[:, kc, n * 512:(n + 1) * 512],
                                             start=(kc == 0), stop=(kc == I_FF // 128 - 1))
                        nc.scalar.activation(y_sc[:, n * 512:(n + 1) * 512], yp[:], AF.Copy,
                                             bias=0.0, scale=wl[:, t3:t3 + 1])
                    nc.gpsimd.indirect_dma_start(
                        out=routed_dram[:],
                        out_offset=bass.IndirectOffsetOnAxis(ap=idl[:, t3:t3 + 1], axis=0),
                        in_=y_sc[:], in_offset=None,
                        bounds_check=bc_s, oob_is_err=False,
                        compute_op=OP.add)

            nc.gpsimd.collective_compute(
                "ReduceScatter", OP.add, replica_groups=rg,
                ins=[routed_dram[:].opt()], outs=[routed_rs[:].opt()])

        # ------------------------------------------------------------------
        # Phase G: shared experts (local tokens) accumulated onto x2
        # ------------------------------------------------------------------
        with contextlib.ExitStack() as ctx:
            pg = ctx.enter_context(tc.tile_pool(name="pg", bufs=2))
            pgw = ctx.enter_context(tc.tile_pool(name="pgw", bufs=2))
            pgp = ctx.enter_context(tc.tile_pool(name="pgp", bufs=1, space="PSUM"))
            pgpT = ctx.enter_context(tc.tile_pool(name="pgpT", bufs=2, space="PSUM"))
            pgpY = ctx.enter_context(tc.tile_pool(name="pgpY", bufs=2, space="PSUM"))

            nc.vector.tensor_copy(out=out_acc[:], in_=x2_sb[:])
            for se in range(NSH):
                guT_sb = pgw.tile([128, KC, 2 * I_FF], BF16, tag="sguT")
                nc.sync.dma_start(guT_sb[:], sgu[se].rearrange("(c p) f -> p c f", p=128))
                dT_sb = pgw.tile([128, I_FF // 128, H], BF16, tag="sdT")
                nc.sync.dma_start(dT_sb[:], sdown[se].rearrange("(c p) f -> p c f", p=128))
                g1 = pgp.tile([128, I_FF], F32, tag="sg1")
                g2 = pgp.tile([128, I_FF], F32, tag="sg2")
                for kc in range(KC):
                    lhsT = h2T_b[:, kc, :]
                    nc.tensor.matmul(g1[:], lhsT, guT_sb[:, kc, 0:I_FF],
                                     start=(kc == 0), stop=(kc == KC - 1))
                    nc.tensor.matmul(g2[:], lhsT, guT_sb[:, kc, I_FF:2 * I_FF],
                                     start=(kc == 0), stop=(kc == KC - 1))
                sig = pg.tile([128, I_FF], F32, tag="ssig")
                nc.scalar.activation(sig[:], g1[:], AF.Sigmoid)
                sil = pg.tile([128, I_FF], F32, tag="ssil")
                nc.vector.tensor_tensor(out=sil[:], in0=sig[:], in1=g1[:], op=OP.mult)
                a_bf = pg.tile([128, I_FF], BF16, tag="sa_bf")
                nc.vector.tensor_tensor(out=a_bf[:], in0=sil[:], in1=g2[:], op=OP.mult)
                aT = pg.tile([128, I_FF // 128, 128], BF16, tag="saT")
                for j in range(I_FF // 128):
                    tp = pgpT.tile([128, 128], BF16, tag="sa_tp")
                    nc.tensor.transpose(tp[:], a_bf[:, j * 128:(j + 1) * 128], identb[:])
                    nc.vector.tensor_copy(out=aT[:, j, :], in_=tp[:])
                for n in range(H // 512):
                    yp = pgpY.tile([128, 512], F32, tag="syp")
                    for kc in range(I_FF // 128):
                        nc.tensor.matmul(yp[:], aT[:, kc, :], dT_sb[:, kc, n * 512:(n + 1) * 512],
                                         start=(kc == 0), stop=(kc == I_FF // 128 - 1))
                    nc.vector.tensor_tensor(out=out_acc[:, n * 512:(n + 1) * 512],
                                            in0=out_acc[:, n * 512:(n + 1) * 512], in1=yp[:], op=OP.add)

        # ------------------------------------------------------------------
        # Phase H: add routed RS result, write output
        # ------------------------------------------------------------------
        with contextlib.ExitStack() as ctx:
            ph = ctx.enter_context(tc.tile_pool(name="ph", bufs=1))
            rb = ph.tile([128, H], BF16, tag="rb")
            nc.sync.dma_start(rb[:], routed_rs[:])
            nc.vector.tensor_tensor(out=out_acc[:], in0=out_acc[:], in1=rb[:], op=OP.add)
            nc.sync.dma_start(out_t[:], out_acc[:])

    return nc


# ----------------------------------------------------------------------------
# Host-side input preparation
# ----------------------------------------------------------------------------
def _prep_inputs(inputs):
    x = np.asarray(inputs["x"], np.float32).reshape(S, H)
    ln1_w = np.asarray(inputs["ln1_w"], np.float32)
    ln1_b = np.asarray(inputs["ln1_b"], np.float32)
    ln2_w = np.asarray(inputs["ln2_w"], np.float32)
    ln2_b = np.asarray(inputs["ln2_b"], np.float32)
    qkv_a_w = np.asarray(inputs["qkv_a_w"], np.float32)
    q_a_ln_w = np.asarray(inputs["q_a_ln_w"], np.float32)
    q_a_ln_b = np.asarray(inputs["q_a_ln_b"], np.float32)
    q_b_w = np.asarray(inputs["q_b_w"], np.float32)
    kv_a_w = np.asarray(inputs["kv_a_w"], np.float32)
    kv_a_ln_w = np.asarray(inputs["kv_a_ln_w"], np.float32)
    kv_a_ln_b = np.asarray(inputs["kv_a_ln_b"], np.float32)
    kv_b_w = np.asarray(inputs["kv_b_w"], np.float32)
    o_w = np.asarray(inputs["o_w"], np.float32)
    gate_w = np.asarray(inputs["gate_w"], np.float32)
    expert_gu = np.asarray(inputs["expert_gu"], np.float32)
    expert_down = np.asarray(inputs["expert_down"], np.float32)
    shared_gu = np.asarray(inputs["shared_gu"], np.float32)
    shared_down = np.asarray(inputs["shared_down"], np.float32)

    # LN affine folding (biases must be zero; true for this module's init)
    for nm, w_, b_ in (("ln1", ln1_w, ln1_b), ("ln2", ln2_w, ln2_b),
                       ("q_a", q_a_ln_w, q_a_ln_b), ("kv_a", kv_a_ln_w, kv_a_ln_b)):
        if not np.allclose(b_, 0.0):
            raise NotImplementedError(f"{nm} bias folding not supported")

    qkv_rows = np.concatenate([qkv_a_w[:QR] * ln1_w[None, :], kv_a_w * ln1_w[None, :]], 0)
    wqkvT = qkv_rows.T.astype(BF)                                  # [H, 2112]
    q_b_f = (q_b_w * q_a_ln_w[None, :]).reshape(NH, QKD, QR)
    kv_b_f = (kv_b_w * kv_a_ln_w[None, :]).reshape(NH, ND + VD, KVR)
    gate_wT = (gate_w * ln2_w[None, :]).T.astype(np.float32)       # [H, E]
    egu_all = (expert_gu * ln2_w[None, :, None]).astype(BF)        # [E, H, 2I]
    sgu_all = (shared_gu * ln2_w[None, :, None]).astype(BF)
    edown_all = expert_down.astype(BF)
    sdown_all = shared_down.astype(BF)

    inv = (10000.0 ** (-np.arange(0, RD, 2, dtype=np.float32) / RD)).astype(np.float32)
    fr = np.outer(np.arange(S, dtype=np.float32), inv)
    cos_t = np.concatenate([np.cos(fr)] * 2, 1).astype(np.float32)  # [S, 64]
    sin_t = np.concatenate([np.sin(fr)] * 2, 1).astype(np.float32)
    cosT = np.ascontiguousarray(cos_t.T)                            # [64, S]
    sinT = np.ascontiguousarray(sin_t.T)

    tri = np.triu(np.ones((128, 128), np.float32))
    ones_m = np.ones((128, 128), np.float32)
    identb = np.eye(128).astype(BF)
    identf = np.eye(128, dtype=np.float32)
    q_idx = np.arange(128)[:, None]
    k_idx = np.arange(128)[None, :]
    cmask = np.where(k_idx <= q_idx, 0.0, NEG).astype(np.float32)
    ids8 = (np.arange(128)[:, None] + 128 * np.arange(S // 128)[None, :]).astype(np.float32)

    in_maps = []
    for c in range(NC_N):
        hsl = slice(HPC * c, HPC * (c + 1))
        esl = slice(EPC * c, EPC * (c + 1))
        emask4 = np.zeros((EPC, 128, E), np.float32)
        for j in range(EPC):
            emask4[j, :, EPC * c + j] = 1.0
        m = {
            "x": np.ascontiguousarray(x[c * TPC:(c + 1) * TPC]),
            "cs_loc": np.ascontiguousarray(
                np.concatenate([cos_t[c * TPC:(c + 1) * TPC], sin_t[c * TPC:(c + 1) * TPC]], 1)),
            "wqb_nT": np.ascontiguousarray(q_b_f[hsl, :ND, :].transpose(2, 0, 1)).astype(BF),
            "wqb_rT": np.ascontiguousarray(q_b_f[hsl, ND:, :].transpose(2, 0, 1)).astype(BF),
            "wkv_kT": np.ascontiguousarray(kv_b_f[hsl, :ND, :].transpose(2, 0, 1)).astype(BF),
            "wkv_v": np.ascontiguousarray(kv_b_f[hsl, ND:, :].transpose(2, 0, 1)).astype(BF),
            "woT": np.ascontiguousarray(o_w.T[c * HPC * VD:(c + 1) * HPC * VD]).astype(BF),
            "egu": np.ascontiguousarray(egu_all[esl]),
            "edown": np.ascontiguousarray(edown_all[esl]),
            "emask4": emask4,
            "wqkvT": wqkvT,
            "cosT": cosT,
            "sinT": sinT,
            "gate_wT": gate_wT,
            "sgu": sgu_all,
            "sdown": sdown_all,
            "tri": tri,
            "ones": ones_m,
            "identb": identb,
            "identf": identf,
            "cmask": cmask,
            "ids": ids8,
        }
        in_maps.append(m)
    return in_maps


_NC_CACHE = {}


def _get_nc():
    if "nc" not in _NC_CACHE:
        _NC_CACHE["nc"] = build_nc()
    return _NC_CACHE["nc"]


def kernel(**inputs) -> np.ndarray:
    nc = _get_nc()
    in_maps = _prep_inputs(inputs)
    res = run_bass_kernel_spmd(nc, in_maps, list(range(NC_N)))
    out = np.concatenate([res.results[c]["out"] for c in range(NC_N)], 0)
    return out.reshape(B, S, H).astype(np.float32)
